# revision 1
# baseline (speedup 1.0000x reference)
"""Trainium2 SPMD kernel for H2OLlama attention (GQA + RoPE + causal softmax + o_proj).

Sharding (8 cores): core = b*4 + g  (b in {0,1} batch, g in {0..3} head group).
Each core handles one batch element, 8 q-heads (g*8..g*8+8) and its 2 kv-heads,
computes QKV projections, RoPE, causal attention, and the row-sharded o_proj
partial product.  Host sums the 4 partials per batch (the o_proj all-reduce)
and transposes back.

All matmuls run in bf16 with fp32 PSUM accumulation; softmax runs in fp32 on
the Scalar engine (exp, no max subtraction -- scores are O(1) here).
"""

import math
import sys

import numpy as np

sys.path.insert(0, "/opt/trn_rl_repo")

import ml_dtypes

import concourse.bass as bass
import concourse.tile as tile
from concourse import bacc, mybir
from concourse.bass_utils import run_bass_kernel_spmd

BF16 = mybir.dt.bfloat16
F32 = mybir.dt.float32

HIDDEN = 4096
N_HEADS = 32
N_KV_HEADS = 8
HEAD_DIM = 128
B, S = 2, 2048
ROPE_THETA = 10000.0

N_CORES = 8
HEADS_PER_CORE = N_HEADS // 4  # 8 q heads per core (4 head groups)
KV_PER_CORE = N_KV_HEADS // 4  # 2 kv heads per core
QDIM = HEADS_PER_CORE * HEAD_DIM  # 1024
KVDIM = KV_PER_CORE * HEAD_DIM  # 256
HC = HIDDEN // 128  # 32 hidden chunks
TT512 = S // 512  # 4 token tiles of 512
TT128 = S // 128  # 16 token tiles of 128
SCALE = 1.0 / math.sqrt(HEAD_DIM)

_BUILD_CACHE = {}


def _build_program():
    nc = bacc.Bacc("TRN2", target_bir_lowering=False, debug=False, num_devices=N_CORES)

    # ---- DRAM I/O ----
    xt_d = nc.dram_tensor("xt", [128, HC, S], BF16, kind="ExternalInput")
    wq_d = nc.dram_tensor("wq", [HEADS_PER_CORE, 128, HC, 128], BF16, kind="ExternalInput")
    wk_d = nc.dram_tensor("wk", [KV_PER_CORE, 128, HC, 128], BF16, kind="ExternalInput")
    wv_d = nc.dram_tensor("wv", [128, HC, KVDIM], BF16, kind="ExternalInput")
    wo_d = nc.dram_tensor("wo", [HC, 128, HEADS_PER_CORE, 128], BF16, kind="ExternalInput")
    cos_d = nc.dram_tensor("cosT", [128, S], F32, kind="ExternalInput")
    sin_d = nc.dram_tensor("sinT", [128, S], F32, kind="ExternalInput")
    rt_d = nc.dram_tensor("rt", [128, 128], BF16, kind="ExternalInput")
    mask_d = nc.dram_tensor("maskb", [128, 1024], BF16, kind="ExternalInput")
    out_d = nc.dram_tensor("out", [HIDDEN, S], BF16, kind="ExternalOutput")

    with tile.TileContext(nc) as tc:
        _kernel_body(nc, tc, xt_d, wq_d, wk_d, wv_d, wo_d, cos_d, sin_d, rt_d, mask_d, out_d)

    nc.compile()
    return nc


def _kernel_body(nc, tc, xt_d, wq_d, wk_d, wv_d, wo_d, cos_d, sin_d, rt_d, mask_d, out_d):
    EXP = mybir.ActivationFunctionType.Exp

    with (
        tc.tile_pool(name="qr", bufs=1) as qr_pool,
        tc.tile_pool(name="kr", bufs=1) as kr_pool,
        tc.tile_pool(name="vv", bufs=1) as v_pool,
        tc.tile_pool(name="aa", bufs=1) as a_pool,
        tc.tile_pool(name="consts", bufs=1) as const_pool,
    ):
        # persistent tensors
        qr = [qr_pool.tile([128, S], BF16, tag=f"qr{h}", name=f"qr{h}") for h in range(HEADS_PER_CORE)]
        kr = [kr_pool.tile([128, S], BF16, tag=f"kr{k}", name=f"kr{k}") for k in range(KV_PER_CORE)]
        vt = v_pool.tile([128, TT128, KVDIM], BF16, tag="v", name="vt")
        at = [a_pool.tile([128, S], BF16, tag=f"a{h}", name=f"a{h}") for h in range(HEADS_PER_CORE)]

        cos_sb = const_pool.tile([128, S], F32, tag="cos", name="cos_sb")
        sin_sb = const_pool.tile([128, S], F32, tag="sin", name="sin_sb")
        rt_sb = const_pool.tile([128, 128], BF16, tag="rt", name="rt_sb")
        mask_sb = const_pool.tile([128, 1024], BF16, tag="mask", name="mask_sb")
        ones_col = const_pool.tile([128, 1], BF16, tag="ones_col", name="ones_col")
        ones_row = const_pool.tile([1, 128], F32, tag="ones_row", name="ones_row")

        nc.sync.dma_start(cos_sb[:], cos_d.ap())
        nc.sync.dma_start(sin_sb[:], sin_d.ap())
        nc.sync.dma_start(rt_sb[:], rt_d.ap())
        nc.sync.dma_start(mask_sb[:], mask_d.ap())
        nc.vector.memset(ones_col[:], 1.0)
        nc.vector.memset(ones_row[:], 1.0)

        # ================= Stage 1+2: projections + RoPE =================
        with (
            tc.tile_pool(name="xt", bufs=1) as x_pool,
            tc.tile_pool(name="wqk", bufs=2) as wqk_pool,
            tc.tile_pool(name="wv", bufs=1) as wv_pool,
            tc.tile_pool(name="qkraw", bufs=3) as qkraw_pool,
            tc.tile_pool(name="ropetmp", bufs=2) as rtmp_pool,
            tc.tile_pool(name="qkps", bufs=2, space="PSUM") as qk_psum,
            tc.tile_pool(name="rotps", bufs=2, space="PSUM") as rot_psum,
            tc.tile_pool(name="vps", bufs=2, space="PSUM") as v_psum,
        ):
            wv_sb = wv_pool.tile([128, HC, KVDIM], BF16, tag="wv", name="wv_sb")
            nc.sync.dma_start(wv_sb[:], wv_d.ap())
            for half in range(2):
                toff = half * 1024
                # load xT for this token half: 2 chunks of 16 hidden-chunks
                xs = []
                for cg in range(2):
                    xtile = x_pool.tile([128, 16, 1024], BF16, tag=f"x{cg}", name=f"x{cg}")
                    nc.sync.dma_start(
                        xtile[:], xt_d.ap()[:, cg * 16 : (cg + 1) * 16, toff : toff + 1024]
                    )
                    xs.append(xtile)

                def xsl(hc, lo, sz):
                    return xs[hc // 16][:, hc % 16, lo : lo + sz]

                # ---- Q^T and K^T projections (+ RoPE) ----
                for do in range(HEADS_PER_CORE + KV_PER_CORE):  # 8 q douts, 2 k douts
                    if do < HEADS_PER_CORE:
                        w_src = wq_d.ap()[do]
                        dst = qr[do]
                    else:
                        w_src = wk_d.ap()[do - HEADS_PER_CORE]
                        dst = kr[do - HEADS_PER_CORE]
                    w_sb = wqk_pool.tile([128, HC, 128], BF16, tag="wqk", name="w_sb")
                    nc.sync.dma_start(w_sb[:], w_src)

                    pss = []
                    for tt in range(2):
                        ps = qk_psum.tile([128, 512], F32, tag=f"qk{tt}", name=f"qkps{tt}")
                        pss.append(ps)
                    for hc in range(HC):
                        for tt in range(2):
                            nc.tensor.matmul(
                                pss[tt][:],
                                lhsT=w_sb[:, hc, :],
                                rhs=xsl(hc, tt * 512, 512),
                                start=(hc == 0),
                                stop=(hc == HC - 1),
                            )
                    # RoPE on the two 512-token tiles
                    for tt in range(2):
                        g0 = toff + tt * 512
                        raw = qkraw_pool.tile([128, 512], BF16, tag="raw", name="raw")
                        nc.scalar.copy(raw[:], pss[tt][:])
                        rot = rot_psum.tile([128, 512], F32, tag="rot", name="rot")
                        nc.tensor.matmul(rot[:], lhsT=rt_sb[:], rhs=raw[:], start=True, stop=True)
                        t1 = rtmp_pool.tile([128, 512], F32, tag="t1", name="t1")
                        nc.vector.tensor_mul(t1[:], raw[:], cos_sb[:, g0 : g0 + 512])
                        t2 = rtmp_pool.tile([128, 512], F32, tag="t2", name="t2")
                        nc.vector.tensor_mul(t2[:], rot[:], sin_sb[:, g0 : g0 + 512])
                        nc.vector.tensor_add(dst[:, g0 : g0 + 512], t1[:], t2[:])

                # ---- V projection (natural [t, d] layout) ----
                for t8 in range(8):
                    j = half * 8 + t8
                    ps = v_psum.tile([128, KVDIM], F32, tag="vps", name="vps")
                    for hc in range(HC):
                        nc.tensor.matmul(
                            ps[:],
                            lhsT=xsl(hc, t8 * 128, 128),
                            rhs=wv_sb[:, hc, :],
                            start=(hc == 0),
                            stop=(hc == HC - 1),
                        )
                    nc.scalar.copy(vt[:, j, :], ps[:])

        # ================= Stage 3: attention =================
        with (
            tc.tile_pool(name="pp", bufs=8) as p_pool,
            tc.tile_pool(name="rinv", bufs=2) as rinv_pool,
            tc.tile_pool(name="sps", bufs=3, space="PSUM") as s_psum,
            tc.tile_pool(name="ops", bufs=2, space="PSUM") as o_psum,
            tc.tile_pool(name="rps", bufs=2, space="PSUM") as r_psum,
            tc.tile_pool(name="bps", bufs=1, space="PSUM") as b_psum,
        ):
            for h in range(HEADS_PER_CORE):
                kvl = h // 4
                for qt in range(TT512):
                    nj = 4 * qt + 4
                    q_rhs = qr[h][:, qt * 512 : (qt + 1) * 512]
                    o_ps = o_psum.tile([128, 512], F32, tag="o", name="o_ps")
                    r_ps = r_psum.tile([1, 512], F32, tag="r", name="r_ps")

                    pts = {}

                    def emit_score(j):
                        s_ps = s_psum.tile([128, 512], F32, tag="s", name="s_ps")
                        nc.tensor.matmul(
                            s_ps[:],
                            lhsT=kr[kvl][:, j * 128 : (j + 1) * 128],
                            rhs=q_rhs,
                            start=True,
                            stop=True,
                        )
                        p_t = p_pool.tile([128, 512], BF16, tag="p", name="p_t")
                        nc.scalar.activation(p_t[:], s_ps[:], EXP, scale=SCALE)
                        if j >= 4 * qt:  # diagonal block: multiplicative causal mask
                            a = j - 4 * qt
                            nc.vector.tensor_mul(
                                p_t[:], p_t[:], mask_sb[:, 512 - a * 128 : 1024 - a * 128]
                            )
                        pts[j] = p_t

                    # software pipeline: scores run 2 iterations ahead of AV
                    emit_score(0)
                    if nj > 1:
                        emit_score(1)
                    for j in range(nj):
                        if j + 2 < nj:
                            emit_score(j + 2)
                        p_t = pts.pop(j)
                        nc.tensor.matmul(
                            o_ps[:],
                            lhsT=vt[:, j, kvl * 128 : (kvl + 1) * 128],
                            rhs=p_t[:],
                            start=(j == 0),
                            stop=(j == nj - 1),
                        )
                        nc.tensor.matmul(
                            r_ps[:],
                            lhsT=ones_col[:],
                            rhs=p_t[:],
                            start=(j == 0),
                            stop=(j == nj - 1),
                        )
                    rinv = rinv_pool.tile([1, 512], F32, tag="rinv", name="rinv")
                    nc.vector.reciprocal(rinv[:], r_ps[:])
                    b_ps = b_psum.tile([128, 512], F32, tag="b", name="b_ps")
                    nc.tensor.matmul(b_ps[:], lhsT=ones_row[:], rhs=rinv[:], start=True, stop=True)
                    o_sb = rinv_pool.tile([128, 512], F32, tag="osb", name="o_sb")
                    nc.vector.tensor_copy(o_sb[:], o_ps[:])
                    nc.vector.tensor_mul(at[h][:, qt * 512 : (qt + 1) * 512], o_sb[:], b_ps[:])

        # ================= Stage 4: o_proj (out^T layout) =================
        with (
            tc.tile_pool(name="wo", bufs=2) as wo_pool,
            tc.tile_pool(name="oout", bufs=4) as out_pool,
            tc.tile_pool(name="outps", bufs=2, space="PSUM") as out_psum,
        ):
            for do in range(HC):  # 32 dout tiles of 128
                wo_sb = wo_pool.tile([128, HEADS_PER_CORE, 128], BF16, tag="wo", name="wo_sb")
                nc.sync.dma_start(wo_sb[:], wo_d.ap()[do])
                pss = [out_psum.tile([128, 512], F32, tag=f"op{tt}", name=f"op{tt}") for tt in range(TT512)]
                for a in range(HEADS_PER_CORE):
                    for tt in range(TT512):
                        nc.tensor.matmul(
                            pss[tt][:],
                            lhsT=wo_sb[:, a, :],
                            rhs=at[a][:, tt * 512 : (tt + 1) * 512],
                            start=(a == 0),
                            stop=(a == HEADS_PER_CORE - 1),
                        )
                for tt in range(TT512):
                    ot = out_pool.tile([128, 512], BF16, tag="ot", name="ot")
                    nc.vector.tensor_copy(ot[:], pss[tt][:])
                    nc.sync.dma_start(
                        out_d.ap()[do * 128 : (do + 1) * 128, tt * 512 : (tt + 1) * 512], ot[:]
                    )


# ======================= host-side sharding =======================


def _rope_tables(position_ids_b):
    pos = position_ids_b.astype(np.float32)  # [S]
    inv_freq = 1.0 / (ROPE_THETA ** (np.arange(0, HEAD_DIM, 2, dtype=np.float32) / HEAD_DIM))
    freqs = pos[:, None] * inv_freq[None, :]  # [S, 64]
    emb = np.concatenate([freqs, freqs], axis=1)  # [S, 128]
    cosT = np.ascontiguousarray(np.cos(emb).T.astype(np.float32))  # [128, S]
    sinT = np.ascontiguousarray(np.sin(emb).T.astype(np.float32))
    return cosT, sinT


def _shared_consts():
    rt = np.zeros((128, 128), dtype=ml_dtypes.bfloat16)
    idx = np.arange(64)
    rt[idx, idx + 64] = 1.0  # RT[j, j+64] = +1  (j < 64)
    rt[idx + 64, idx] = -1.0  # RT[j+64, j] = -1
    maskb = np.zeros((128, 1024), dtype=ml_dtypes.bfloat16)
    k = np.arange(128)[:, None]
    c = np.arange(1024)[None, :]
    maskb[:] = (c >= k + 512).astype(ml_dtypes.bfloat16)
    return rt, maskb


def kernel(hidden_states, position_ids, Wq, Wk, Wv, Wo):
    bf16 = ml_dtypes.bfloat16
    if "nc" not in _BUILD_CACHE:
        _BUILD_CACHE["nc"] = _build_program()
    nc = _BUILD_CACHE["nc"]

    rt, maskb = _shared_consts()
    Wq16, Wk16, Wv16, Wo16 = (w.astype(bf16) for w in (Wq, Wk, Wv, Wo))

    xts, coss, sins = [], [], []
    for b in range(B):
        xb = np.asarray(hidden_states[b], dtype=np.float32).T.astype(bf16)  # [4096, S]
        xt = np.ascontiguousarray(xb.reshape(HC, 128, S).transpose(1, 0, 2))  # [128, 32, S]
        xts.append(xt)
        cosT, sinT = _rope_tables(np.asarray(position_ids[b]))
        coss.append(cosT)
        sins.append(sinT)

    in_maps = []
    for core in range(N_CORES):
        b, g = core // 4, core % 4
        wq = np.ascontiguousarray(
            Wq16[:, g * QDIM : (g + 1) * QDIM].reshape(HC, 128, HEADS_PER_CORE, 128).transpose(2, 1, 0, 3)
        )
        wk = np.ascontiguousarray(
            Wk16[:, g * KVDIM : (g + 1) * KVDIM].reshape(HC, 128, KV_PER_CORE, 128).transpose(2, 1, 0, 3)
        )
        wv = np.ascontiguousarray(
            Wv16[:, g * KVDIM : (g + 1) * KVDIM].reshape(HC, 128, KVDIM).transpose(1, 0, 2)
        )
        wo = np.ascontiguousarray(
            Wo16[g * QDIM : (g + 1) * QDIM, :].reshape(HEADS_PER_CORE, 128, HC, 128).transpose(2, 1, 0, 3)
        )
        in_maps.append(
            {
                "xt": xts[b],
                "wq": wq,
                "wk": wk,
                "wv": wv,
                "wo": wo,
                "cosT": coss[b],
                "sinT": sins[b],
                "rt": rt,
                "maskb": maskb,
            }
        )

    res = run_bass_kernel_spmd(nc, in_maps, list(range(N_CORES))).results

    out = np.empty((B, S, HIDDEN), dtype=np.float32)
    for b in range(B):
        acc = res[4 * b]["out"].astype(np.float32)
        for g in range(1, 4):
            acc = acc + res[4 * b + g]["out"]
        out[b] = acc.T
    return out

